# revision 22
# baseline (speedup 1.0000x reference)
"""AdaAT (per-channel affine warp + bilinear grid_sample) on 8 TRN2 NeuronCores.

Sharding: data-parallel over batch (B=8 -> 1 sample per core).

v5: the ap_gather ucode wall (~27ns/index-slot) is halved by PAIR-SHARING:
vertically-adjacent output pixels (2v, w) and (2v+1, w) share ONE gather
index (the elementwise-min "anchor" of their floored source coords).  Each
gathered 4-byte lane value is a bf16-packed horizontal pair
(bf16 s[j], bf16 s[j+1]), so one lane covers both x-taps of a pixel.  The
16 lanes of each channel group hold the packed source image shifted by
128*r + c for (r, c) in 4 rows x 3 cols -- a patch that covers every
(row-tap, x-pair) offset either pixel of the pair can need, since the
per-pixel affine steps satisfy |ax|,|bx| <= 2 (data max ~1.58; holds for
any scale <= 1.984).  Per-pixel offsets into the patch are resolved
post-gather on the DVE with is_equal-masked sums over the 12 lanes
(bf16 ops, exact for {0,1}-masks), then bilinear-combined with f32 fracs.

Index slots per core: 32 calls x 8192 pairs = 262144 (vs 524288) ->
gather ~7.1ms.  All DMA (12-lane source replication ~6.2MB/call, lane
extraction ~3MB/call, output) is spread across the sync/scalar/tensor/
vector HWDGE rings to hide under the gather stream.
"""

import numpy as np

B, D, H, W = 8, 256, 128, 128
NPIX = H * W  # 16384
NCORES = 8
PI = 3.14159  # matches reference
CALLS = D // 8  # 32 calls per core, 8 channels each
NPAIR = NPIX // 2  # 8192 pair-slots per channel per call
NCHUNK = 4
CHUNK = NPAIR // NCHUNK  # 2048 slots per ap_gather
SC = 2  # DVE super-chunks per call (2 gather chunks each)
SCW = NPAIR // SC // 16  # 256: free width of per-sc tiles
PAD = 512
CH_PITCHB = NPIX + PAD      # per-channel pitch in bf16 elements
CH_PITCHF = CH_PITCHB // 2  # ... in f32 words
LANES = [(r, c) for r in range(4) for c in range(4)]  # all 16 lanes

_GRAPH_CACHE = {}


def _host_constants():
    p = np.arange(128)
    s = np.arange(512)
    # wrapped-16 pair iotas: pair slot q = s*16 + p%16; v = q//128, w = q%128
    q = s[None, :] * 16 + (p[:, None] % 16)  # [128, 512]
    iww = (q % 128).astype(np.float32)
    ihw = (2 * (q // 128)).astype(np.float32)
    # P_rc-layout iotas: partition 16g+r, f: v = (f//128)*16 + r, w = f%128
    f = np.arange(512)
    iwp = np.broadcast_to((f % 128).astype(np.float32), (128, 512)).copy()
    ihp = (2 * ((f[None, :] // 128) * 16 + (p[:, None] % 16))).astype(np.float32)
    sel_lhsT = np.zeros((128, 256), dtype=np.float32)
    selmask = np.zeros((128, 64), dtype=np.float32)
    for chunk in range(2):
        c = chunk * 128 + np.arange(128)
        sel_lhsT[:, chunk * 128 : chunk * 128 + 128] = (
            (c[:, None] % 8) == (p[None, :] // 16)
        ).astype(np.float32)
        selmask[:, chunk * 32 : chunk * 32 + 32] = (
            (c[:, None] // 8) == np.arange(32)[None, :]
        ).astype(np.float32)
    return dict(iww=iww, ihw=ihw, iwp=iwp, ihp=ihp,
                sel_lhsT=sel_lhsT, selmask=selmask)


def _col2(x):
    return np.ascontiguousarray(x.reshape(2, 128).T)


def _mm_layout(Wm, n_out):
    return np.ascontiguousarray(
        Wm.reshape(2, 128, n_out).transpose(1, 0, 2).reshape(128, 2 * n_out)
    )


def _build(trace_label=""):
    import concourse.bass as bass
    import concourse.tile as tile
    from concourse import bacc, mybir
    from concourse.bass import ds

    f32, i32, i16 = mybir.dt.float32, mybir.dt.int32, mybir.dt.int16
    bf16 = mybir.dt.bfloat16
    AF = mybir.ActivationFunctionType
    OP = mybir.AluOpType

    nc = bacc.Bacc("TRN2", target_bir_lowering=False, debug=False,
                   num_devices=NCORES)

    def din(name, shape):
        return nc.dram_tensor(name, list(shape), f32, kind="ExternalInput").ap()

    fmb = din("fmb", [D * CH_PITCHF])
    pc = din("pc", [128, 2])
    w1 = din("w1", [128, 512])
    ws = din("ws", [128, 512])
    wr = din("wr", [128, 512])
    wt = din("wt", [128, 1024])
    b1 = din("b1", [128, 2])
    bs = din("bs", [128, 2])
    br = din("br", [128, 2])
    bt = din("bt", [128, 4])
    iww_d = din("iww", [128, 512])
    ihw_d = din("ihw", [128, 512])
    iwp_d = din("iwp", [128, 512])
    ihp_d = din("ihp", [128, 512])
    sel_lhsT_d = din("sel_lhsT", [128, 256])
    selmask_d = din("selmask", [128, 64])
    out_d = nc.dram_tensor("out", [D * NPIX], f32, kind="ExternalOutput").ap()

    fmb_b = fmb.bitcast(bf16).rearrange("(G q) -> G q", G=D)  # [256, 16896]
    out5 = out_d.rearrange("(G c r x w) -> G c r x w",
                           G=D, c=4, r=16, x=2, w=128)

    with tile.TileContext(nc) as tc:
        with (
            tc.tile_pool(name="persist", bufs=1) as persist,
            tc.tile_pool(name="psum", bufs=1, space="PSUM") as psum,
            tc.tile_pool(name="src", bufs=2) as srcp,
            tc.tile_pool(name="gath", bufs=2) as gathp,
            tc.tile_pool(name="bi16", bufs=2) as bi16p,
            tc.tile_pool(name="prc", bufs=2) as prcp,
            tc.tile_pool(name="outp", bufs=2) as outp,
            tc.tile_pool(name="ws", bufs=1) as wsp,
            tc.tile_pool(name="widx", bufs=1) as widxp,
        ):
            # ---- staging ----
            def stage_p(ap_dram, shape, tag):
                t = persist.tile(list(shape), f32, tag=tag)
                nc.sync.dma_start(t[:], ap_dram[:])
                return t

            pc_t = stage_p(pc, [128, 2], "s_pc")
            b1_t = stage_p(b1, [128, 2], "s_b1")
            bs_t = stage_p(bs, [128, 2], "s_bs")
            br_t = stage_p(br, [128, 2], "s_br")
            bt_t = stage_p(bt, [128, 4], "s_bt")
            selmask_t = stage_p(selmask_d, [128, 64], "s_selmask")
            # bf16 iotas (values 0..254, exact); gpsimd DMA casts f32->bf16
            iww_t = persist.tile([128, 512], bf16, tag="s_iww")
            nc.gpsimd.dma_start(iww_t[:], iww_d[:])
            ihw_t = persist.tile([128, 512], bf16, tag="s_ihw")
            nc.gpsimd.dma_start(ihw_t[:], ihw_d[:])
            iwp_t = persist.tile([128, 512], bf16, tag="s_iwp")
            nc.gpsimd.dma_start(iwp_t[:], iwp_d[:])
            ihp_t = persist.tile([128, 512], bf16, tag="s_ihp")
            nc.gpsimd.dma_start(ihp_t[:], ihp_d[:])
            # transient weights -> main-loop tags (dead after setup)
            w1_t = gathp.tile([128, 512], f32, tag="G")
            nc.sync.dma_start(w1_t[:], w1[:])
            ws_t = widxp.tile([128, 512], f32, tag="wa")
            nc.sync.dma_start(ws_t[:], ws[:])
            wr_t = widxp.tile([128, 512], f32, tag="wb")
            nc.sync.dma_start(wr_t[:], wr[:])
            wt_t = srcp.tile([128, 1024], f32, tag="S")
            nc.sync.dma_start(wt_t[:], wt[:])
            sel_lhsT_t = widxp.tile([128, 256], f32, tag="wc")
            nc.sync.dma_start(sel_lhsT_t[:], sel_lhsT_d[:])

            # ---- param MLP in column layout ----
            p_sb = persist.tile([128, 2], f32)

            def mlp_cols(w_tile, rhs_tile, bias_tile, n_chunks_out, func,
                         out_tile, scale=1.0, n_out_cols=256):
                for m in range(n_chunks_out):
                    ps = psum.tile([128, 1], f32, space="PSUM")
                    for kk in range(2):
                        nc.tensor.matmul(
                            ps[:],
                            lhsT=w_tile[:, kk * n_out_cols + m * 128 :
                                        kk * n_out_cols + m * 128 + 128],
                            rhs=rhs_tile[:, kk : kk + 1],
                            start=(kk == 0), stop=(kk == 1),
                        )
                    nc.scalar.activation(out_tile[:, m : m + 1], ps[:], func,
                                         bias=bias_tile[:, m : m + 1],
                                         scale=scale)

            mlp_cols(w1_t, pc_t, b1_t, 2, AF.Relu, p_sb)
            sig_sb = persist.tile([128, 2], f32)
            mlp_cols(ws_t, p_sb, bs_t, 2, AF.Sigmoid, sig_sb)
            tnh_sb = persist.tile([128, 2], f32)
            mlp_cols(wr_t, p_sb, br_t, 2, AF.Tanh, tnh_sb)
            tt_sb = persist.tile([128, 4], f32)
            mlp_cols(wt_t, p_sb, bt_t, 4, AF.Tanh, tt_sb, n_out_cols=512)

            zero_b = persist.tile([128, 1], f32)
            nc.vector.memset(zero_b[:], 0.0)
            cs_sb = persist.tile([128, 2], f32)
            sn_sb = persist.tile([128, 2], f32)
            sh_sb = persist.tile([128, 2], f32)
            for m in range(2):
                # sin LUT is only accurate on ~[-pi, pi]; cos via half-angle
                nc.scalar.activation(sn_sb[:, m : m + 1], tnh_sb[:, m : m + 1],
                                     AF.Sin, bias=zero_b[:], scale=PI)
                nc.scalar.activation(sh_sb[:, m : m + 1], tnh_sb[:, m : m + 1],
                                     AF.Sin, bias=zero_b[:], scale=PI / 2.0)
                nc.scalar.activation(sh_sb[:, m : m + 1], sh_sb[:, m : m + 1],
                                     AF.Square, bias=zero_b[:], scale=1.0)
                nc.vector.tensor_scalar(cs_sb[:, m : m + 1],
                                        sh_sb[:, m : m + 1], -2.0, 1.0,
                                        op0=OP.mult, op1=OP.add)

            # ---- affine coefficients (pixel space) ----
            P10 = persist.tile([128, 10], f32)
            AXF = 256.0 / 127.0
            for m in range(2):
                o = m * 5
                csig = persist.tile([128, 1], f32, tag="csig")
                ssig = persist.tile([128, 1], f32, tag="ssig")
                nc.vector.tensor_tensor(csig[:], cs_sb[:, m : m + 1],
                                        sig_sb[:, m : m + 1], op=OP.mult)
                nc.vector.tensor_tensor(ssig[:], sn_sb[:, m : m + 1],
                                        sig_sb[:, m : m + 1], op=OP.mult)
                nc.vector.tensor_scalar(P10[:, o + 2 : o + 3], csig[:], AXF,
                                        None, op0=OP.mult)
                nc.vector.tensor_scalar(P10[:, o : o + 1], ssig[:], -AXF,
                                        None, op0=OP.mult)
                nc.vector.tensor_scalar(P10[:, o + 4 : o + 5], ssig[:], AXF,
                                        None, op0=OP.mult)  # bxn = -bx
                e1 = persist.tile([128, 1], f32, tag="e1")
                nc.vector.tensor_scalar(e1[:], tt_sb[:, m : m + 1], 64.0, 63.5,
                                        op0=OP.mult, op1=OP.add)
                e2 = persist.tile([128, 1], f32, tag="e2")
                nc.vector.scalar_tensor_tensor(e2[:], csig[:], -128.0, e1[:],
                                               op0=OP.mult, op1=OP.add)
                nc.vector.scalar_tensor_tensor(P10[:, o + 1 : o + 2], ssig[:],
                                               128.0, e2[:],
                                               op0=OP.mult, op1=OP.add)
                f1 = persist.tile([128, 1], f32, tag="f1")
                nc.vector.tensor_scalar(f1[:], tt_sb[:, m + 2 : m + 3], 64.0,
                                        63.5, op0=OP.mult, op1=OP.add)
                f2 = persist.tile([128, 1], f32, tag="f2")
                nc.vector.scalar_tensor_tensor(f2[:], ssig[:], -128.0, f1[:],
                                               op0=OP.mult, op1=OP.add)
                nc.vector.scalar_tensor_tensor(P10[:, o + 3 : o + 4], csig[:],
                                               -128.0, f2[:],
                                               op0=OP.mult, op1=OP.add)

            # ---- grouped coefficient tables [128, 32] ----
            grp = {}
            for name, t_idx in (("ax", 2), ("bx", 0), ("ex", 1), ("ey", 3),
                                ("bxn", 4)):
                g_ps = psum.tile([128, 32], f32, space="PSUM")
                for m in range(2):
                    rhs = persist.tile([128, 32], f32, tag="grp_rhs")
                    nc.vector.tensor_scalar(
                        rhs[:], selmask_t[:, m * 32 : m * 32 + 32],
                        P10[:, m * 5 + t_idx : m * 5 + t_idx + 1], None,
                        op0=OP.mult)
                    nc.tensor.matmul(g_ps[:],
                                     lhsT=sel_lhsT_t[:, m * 128 : m * 128 + 128],
                                     rhs=rhs[:], start=(m == 0), stop=(m == 1))
                g_sb = persist.tile([128, 32], f32, tag=f"grp_{name}")
                nc.vector.tensor_copy(g_sb[:], g_ps[:])
                grp[name] = g_sb

            # ---- per-pair anchor gather index (wrapped layout) for call j ----
            def compute_idx(j):
                kk = ds(j, 1)
                ax_c, bx_c = grp["ax"][:, kk], grp["bx"][:, kk]
                bxn_c, ex_c = grp["bxn"][:, kk], grp["ex"][:, kk]
                ey_c = grp["ey"][:, kk]

                def coord_floors(iw_t, ih_t, sa, sb, se, te_tag, to_tag):
                    te = widxp.tile([128, 512], f32, tag=te_tag, name="te")
                    to = widxp.tile([128, 512], f32, tag=to_tag, name="to")
                    fr = widxp.tile([128, 512], f32, tag="wsc", name="fr")
                    nc.vector.tensor_scalar(te[:], iw_t[:], sa, None,
                                            op0=OP.mult)
                    nc.vector.scalar_tensor_tensor(te[:], ih_t[:], sb, te[:],
                                                   op0=OP.mult, op1=OP.add)
                    nc.vector.tensor_scalar(te[:], te[:], se, None, op0=OP.add)
                    nc.vector.tensor_scalar(to[:], te[:], sb, None, op0=OP.add)
                    for q in (te, to):
                        nc.vector.tensor_scalar(q[:], q[:], 0.0, 127.0,
                                                op0=OP.max, op1=OP.min)
                        # floor via round-magic: fr=round(q); fr=q-fr (+1 if
                        # negative) = frac; q -= frac
                        nc.vector.tensor_scalar(fr[:], q[:], 12582912.0,
                                                -12582912.0,
                                                op0=OP.add, op1=OP.add)
                        nc.vector.tensor_tensor(fr[:], q[:], fr[:],
                                                op=OP.subtract)
                        nc.vector.scalar_tensor_tensor(fr[:], fr[:], 0.0,
                                                       fr[:], op0=OP.is_lt,
                                                       op1=OP.add)
                        nc.vector.tensor_tensor(q[:], q[:], fr[:],
                                                op=OP.subtract)
                    return te, to

                x0e, x0o = coord_floors(iww_t, ihw_t, ax_c, bx_c, ex_c,
                                        "wa", "wb")
                nc.vector.tensor_tensor(x0e[:], x0e[:], x0o[:], op=OP.min)
                # fl = floor(axm/2); block index jb = aym*64 + fl
                fl = widxp.tile([128, 512], f32, tag="wb", name="fl")
                fr2 = widxp.tile([128, 512], f32, tag="wsc", name="fr2")
                nc.vector.tensor_scalar(fl[:], x0e[:], 0.5, None, op0=OP.mult)
                nc.vector.tensor_scalar(fr2[:], fl[:], 12582912.0, -12582912.0,
                                        op0=OP.add, op1=OP.add)
                nc.vector.tensor_tensor(fr2[:], fl[:], fr2[:], op=OP.subtract)
                nc.vector.scalar_tensor_tensor(fr2[:], fr2[:], 0.0, fr2[:],
                                               op0=OP.is_lt, op1=OP.add)
                nc.vector.tensor_tensor(fl[:], fl[:], fr2[:], op=OP.subtract)
                y0e, y0o = coord_floors(iww_t, ihw_t, bxn_c, ax_c, ey_c,
                                        "wa", "wc")
                nc.vector.tensor_tensor(y0e[:], y0e[:], y0o[:], op=OP.min)
                nc.vector.scalar_tensor_tensor(fl[:], y0e[:], 64.0, fl[:],
                                               op0=OP.mult, op1=OP.add)
                b32 = widxp.tile([128, 512], i32, tag="wc", name="b32")
                nc.vector.tensor_copy(b32[:], fl[:])
                bt_ = bi16p.tile([128, 512], i16, tag="bi16")
                nc.vector.tensor_copy(bt_[:], b32[:])
                return bt_

            bi16_tiles = [None] * (CALLS + 1)
            bi16_tiles[0] = compute_idx(0)

            x_engs = [nc.scalar, nc.sync]

            # 16-lane bf16-singles source load for call j; the first 11 lanes
            # ride the SWDGE (gpsimd) queue -- dispatched one call AHEAD so
            # their Pool-queue slot lands between gather streams.
            def load_S(j):
                S = srcp.tile([128, NPIX // 2], f32, tag="S", name="S")
                Sb = S[:].bitcast(bf16)
                for li, (r, cx) in enumerate(LANES):
                    sig = 128 * r + cx
                    eng = (nc.gpsimd if li < 11
                           else (nc.sync if li < 13 else nc.scalar))
                    eng.dma_start(
                        Sb[li::16, :],
                        fmb_b[ds(8 * j, 8), ds(sig, NPIX)],
                        max_dma_last_dim=8192,
                    )
                return S

            S_tiles = [None] * CALLS
            S_tiles[0] = load_S(0)

            # ================= main loop =================
            for k in range(CALLS):
                kk = ds(k, 1)
                if k + 1 < CALLS:
                    S_tiles[k + 1] = load_S(k + 1)

                # ---- indices for the NEXT call (gather k+1 never waits) ----
                if k + 1 < CALLS:
                    bi16_tiles[k + 1] = compute_idx(k + 1)
                S = S_tiles[k]

                bi16 = bi16_tiles[k]
                ax_c, bx_c = grp["ax"][:, kk], grp["bx"][:, kk]
                bxn_c, ex_c = grp["bxn"][:, kk], grp["ex"][:, kk]
                ey_c = grp["ey"][:, kk]

                for sc in range(SC):
                    P = [prcp.tile([128, SCW], f32, tag=f"P{l}",
                                   name=f"P{l}")
                         for l in range(16)]
                    # ---- gather 2 chunks + lane extraction ----
                    for cc in range(2):
                        c = sc * 2 + cc
                        G = gathp.tile([128, CHUNK], f32, tag="G")
                        nc.gpsimd.ap_gather(
                            G[:].bitcast(bf16), S[:].bitcast(bf16),
                            bi16[:, ds(c * 128, 128)],
                            channels=128, num_elems=NPIX // 2, d=2,
                            num_idxs=CHUNK)
                        for li in range(16):
                            x_engs[li % 2].dma_start(
                                P[li][:, ds(cc * 128, 128)],
                                G[:][li::16, :]
                                .rearrange("g (r w) -> g r w", r=16),
                            )

                    # ---- per-pixel coords/fracs/offsets in P_rc layout ----
                    ss = ds(sc * SCW, SCW)
                    iwp_s, ihp_s = iwp_t[:, ss], ihp_t[:, ss]

                    def coord_full(sa, sb, se, tags):
                        te = wsp.tile([128, SCW], f32, tag=tags[0])
                        to = wsp.tile([128, SCW], f32, tag=tags[1])
                        fe = wsp.tile([128, SCW], f32, tag=tags[2])
                        fo = wsp.tile([128, SCW], f32, tag=tags[3])
                        nc.vector.tensor_scalar(te[:], iwp_s, sa, None,
                                                op0=OP.mult)
                        nc.vector.scalar_tensor_tensor(te[:], ihp_s, sb, te[:],
                                                       op0=OP.mult, op1=OP.add)
                        nc.vector.tensor_scalar(te[:], te[:], se, None,
                                                op0=OP.add)
                        nc.vector.tensor_scalar(to[:], te[:], sb, None,
                                                op0=OP.add)
                        for q, fq in ((te, fe), (to, fo)):
                            nc.vector.tensor_scalar(q[:], q[:], 0.0, 127.0,
                                                    op0=OP.max, op1=OP.min)
                            nc.vector.tensor_scalar(fq[:], q[:], 12582912.0,
                                                    -12582912.0,
                                                    op0=OP.add, op1=OP.add)
                            nc.vector.tensor_tensor(fq[:], q[:], fq[:],
                                                    op=OP.subtract)
                            nc.vector.scalar_tensor_tensor(fq[:], fq[:], 0.0,
                                                           fq[:],
                                                           op0=OP.is_lt,
                                                           op1=OP.add)
                            nc.vector.tensor_tensor(q[:], q[:], fq[:],
                                                    op=OP.subtract)
                        return te, to, fe, fo

                    x0e, x0o, fxe, fxo = coord_full(
                        ax_c, bx_c, ex_c, ("px0", "px1", "pfx0", "pfx1"))
                    y0e, y0o, fye, fyo = coord_full(
                        bxn_c, ax_c, ey_c, ("py0", "py1", "pfy0", "pfy1"))
                    axm = wsp.tile([128, SCW], f32, tag="paxm")
                    aym = wsp.tile([128, SCW], f32, tag="paym")
                    nc.vector.tensor_tensor(axm[:], x0e[:], x0o[:], op=OP.min)
                    nc.vector.tensor_tensor(aym[:], y0e[:], y0o[:], op=OP.min)
                    # par = axm mod 2 (block-misalignment of the d=2 gather)
                    par = wsp.tile([128, SCW], f32, tag="ppar")
                    pt = wsp.tile([128, SCW], f32, tag="ppt")
                    nc.vector.tensor_scalar(pt[:], axm[:], 0.5, None,
                                            op0=OP.mult)
                    nc.vector.tensor_scalar(par[:], pt[:], 12582912.0,
                                            -12582912.0, op0=OP.add,
                                            op1=OP.add)
                    nc.vector.tensor_tensor(par[:], pt[:], par[:],
                                            op=OP.subtract)
                    nc.vector.scalar_tensor_tensor(par[:], par[:], 0.0,
                                                   par[:], op0=OP.is_lt,
                                                   op1=OP.add)
                    nc.vector.tensor_scalar(par[:], par[:], 2.0, None,
                                            op0=OP.mult)
                    # deltas: dx in 0..2 (+par -> 0..3), dy in 0..2
                    dups = []
                    for dn, (pos, anc, addpar) in enumerate(
                            ((x0e, axm, True), (x0o, axm, True),
                             (y0e, aym, False), (y0o, aym, False))):
                        nc.vector.tensor_tensor(pos[:], pos[:], anc[:],
                                                op=OP.subtract)
                        if addpar:
                            nc.vector.tensor_tensor(pos[:], pos[:], par[:],
                                                    op=OP.add)
                        dd = wsp.tile([128, 2 * SCW], bf16, tag=f"dd{dn}")
                        nc.vector.tensor_copy(dd[:, 0::2], pos[:])
                        nc.vector.tensor_copy(dd[:, 1::2], pos[:])
                        dups.append(dd)
                    dxe, dxo, dye, dyo = dups

                    Lb = [P[l][:].bitcast(bf16) for l in range(16)]

                    # ---- masked patch selection + bilinear, per class ----
                    for cls, (dxd, dyd, fx, fy, par) in enumerate(
                            ((dxe, dye, fxe, fye, 0),
                             (dxo, dyo, fxo, fyo, 1))):
                        Cr = []
                        Ct = wsp.tile([128, 2 * SCW], bf16, tag="paxm",
                                      name="Ct")
                        ctags = ("px0", "px1", "py0", "py1")
                        for r in range(4):
                            C = wsp.tile([128, 2 * SCW], bf16, tag=ctags[r],
                                         name=f"C{r}")
                            nc.vector.scalar_tensor_tensor(
                                C[:], dxd[:], 0.0, Lb[r * 4 + 0],
                                op0=OP.is_equal, op1=OP.mult)
                            for cx in (1, 2, 3):
                                nc.vector.scalar_tensor_tensor(
                                    Ct[:], dxd[:], float(cx), Lb[r * 4 + cx],
                                    op0=OP.is_equal, op1=OP.mult)
                                nc.vector.tensor_tensor(C[:], C[:], Ct[:],
                                                        op=OP.add)
                            Cr.append(C)
                        PK = []
                        ktags = ("paym", "K1")
                        for tap in range(2):
                            Kt = wsp.tile([128, 2 * SCW], bf16,
                                          tag=ktags[tap], name=f"K{tap}")
                            nc.vector.scalar_tensor_tensor(
                                Kt[:], dyd[:], 0.0, Cr[tap][:],
                                op0=OP.is_equal, op1=OP.mult)
                            for rr in (1, 2):
                                nc.vector.scalar_tensor_tensor(
                                    Ct[:], dyd[:], float(rr), Cr[tap + rr][:],
                                    op0=OP.is_equal, op1=OP.mult)
                                nc.vector.tensor_tensor(Kt[:], Kt[:], Ct[:],
                                                        op=OP.add)
                            PK.append(Kt)
                        # x-lerp from packed bf16 pairs, then y-lerp
                        Rs = []
                        for tap in range(2):
                            lo = PK[tap][:, 0::2]
                            hi = PK[tap][:, 1::2]
                            Rt = wsp.tile([128, SCW], f32, tag=f"R{tap}")
                            nc.vector.tensor_tensor(Rt[:], hi, lo,
                                                    op=OP.subtract)
                            nc.vector.tensor_tensor(Rt[:], Rt[:], fx[:],
                                                    op=OP.mult)
                            nc.vector.tensor_tensor(Rt[:], Rt[:], lo,
                                                    op=OP.add)
                            Rs.append(Rt)
                        O = outp.tile([128, SCW], f32, tag=f"O{cls}")
                        nc.vector.tensor_tensor(Rs[1][:], Rs[1][:], Rs[0][:],
                                                op=OP.subtract)
                        nc.vector.tensor_tensor(Rs[1][:], Rs[1][:], fy[:],
                                                op=OP.mult)
                        nc.vector.tensor_tensor(O[:], Rs[1][:], Rs[0][:],
                                                op=OP.add)
                        # ---- output DMA: rows h = 2v+par, contiguous w ----
                        for cc in range(2):
                            x_engs[(cls + cc) % 2].dma_start(
                                out5[ds(8 * k, 8), sc * 2 + cc,
                                     slice(None), par, slice(None)],
                                O[:, ds(cc * 128, 128)],
                            )

    nc.compile()
    return nc


def _prepare_in_maps(feature_map, para_code, W1, b1, Ws, bs, Wr, br, Wt, bt):
    import ml_dtypes

    consts = _host_constants()
    Wt_re = np.concatenate([Wt[:, 0::2], Wt[:, 1::2]], axis=1)
    bt_re = np.concatenate([bt[0::2], bt[1::2]])
    common = dict(
        w1=_mm_layout(W1, 256), ws=_mm_layout(Ws, 256), wr=_mm_layout(Wr, 256),
        wt=_mm_layout(Wt_re, 512),
        b1=_col2(b1), bs=_col2(bs), br=_col2(br),
        bt=np.ascontiguousarray(bt_re.reshape(4, 128).T),
        **consts,
    )
    common = {k: np.ascontiguousarray(v, dtype=np.float32)
              for k, v in common.items()}
    in_maps = []
    for i in range(NCORES):
        flat = np.ascontiguousarray(feature_map[i].reshape(D, NPIX),
                                    dtype=np.float32)
        fmb = np.zeros((D, CH_PITCHB), dtype=ml_dtypes.bfloat16)
        fmb[:, :NPIX] = flat.astype(ml_dtypes.bfloat16)
        m = dict(common)
        m["fmb"] = np.ascontiguousarray(fmb).view(np.float32).reshape(-1)
        m["pc"] = _col2(para_code[i])
        in_maps.append(m)
    return in_maps


def _run(inputs, trace=False):
    from concourse.bass_utils import run_bass_kernel_spmd

    if "nc" not in _GRAPH_CACHE:
        _GRAPH_CACHE["nc"] = _build()
    nc = _GRAPH_CACHE["nc"]
    in_maps = _prepare_in_maps(**inputs)
    res = run_bass_kernel_spmd(nc, in_maps, core_ids=list(range(NCORES)),
                               trace=trace)
    out = np.stack([
        np.asarray(res.results[i]["out"]).reshape(D, H, W)
        for i in range(NCORES)
    ])
    return out, res


def kernel(**inputs) -> np.ndarray:
    out, _ = _run(inputs, trace=False)
    return out


# revision 26
# speedup vs baseline: 1.1841x; 1.1841x over previous
"""AdaAT (per-channel affine warp + bilinear grid_sample) on 8 TRN2 NeuronCores.

Sharding: data-parallel over batch (B=8 -> 1 sample per core).

v5: the ap_gather ucode wall (~27ns/index-slot) is halved by PAIR-SHARING:
vertically-adjacent output pixels (2v, w) and (2v+1, w) share ONE gather
index (the elementwise-min "anchor" of their floored source coords).  Each
gathered 4-byte lane value is a bf16-packed horizontal pair
(bf16 s[j], bf16 s[j+1]), so one lane covers both x-taps of a pixel.  The
16 lanes of each channel group hold the packed source image shifted by
128*r + c for (r, c) in 4 rows x 3 cols -- a patch that covers every
(row-tap, x-pair) offset either pixel of the pair can need, since the
per-pixel affine steps satisfy |ax|,|bx| <= 2 (data max ~1.58; holds for
any scale <= 1.984).  Per-pixel offsets into the patch are resolved
post-gather on the DVE with is_equal-masked sums over the 12 lanes
(bf16 ops, exact for {0,1}-masks), then bilinear-combined with f32 fracs.

Index slots per core: 32 calls x 8192 pairs = 262144 (vs 524288) ->
gather ~7.1ms.  All DMA (12-lane source replication ~6.2MB/call, lane
extraction ~3MB/call, output) is spread across the sync/scalar/tensor/
vector HWDGE rings to hide under the gather stream.
"""

import numpy as np

B, D, H, W = 8, 256, 128, 128
NPIX = H * W  # 16384
NCORES = 8
PI = 3.14159  # matches reference
CALLS = D // 8  # 32 calls per core, 8 channels each
NPAIR = NPIX // 2  # 8192 pair-slots per channel per call
NCHUNK = 4
CHUNK = NPAIR // NCHUNK  # 2048 slots per ap_gather
SC = 2  # DVE super-chunks per call (2 gather chunks each)
SCW = NPAIR // SC // 16  # 256: free width of per-sc tiles
PAD = 512
CH_PITCHB = NPIX + PAD      # per-channel pitch in bf16 elements
CH_PITCHF = CH_PITCHB // 2  # ... in f32 words
LANES = [(r, c) for r in range(4) for c in range(4)]  # all 16 lanes

_GRAPH_CACHE = {}


def _host_constants():
    p = np.arange(128)
    s = np.arange(512)
    # wrapped-16 pair iotas: pair slot q = s*16 + p%16; v = q//128, w = q%128
    q = s[None, :] * 16 + (p[:, None] % 16)  # [128, 512]
    iww = (q % 128).astype(np.float32)
    ihw = (2 * (q // 128)).astype(np.float32)
    # P_rc-layout iotas: partition 16g+r, f: v = (f//128)*16 + r, w = f%128
    f = np.arange(512)
    iwp = np.broadcast_to((f % 128).astype(np.float32), (128, 512)).copy()
    ihp = (2 * ((f[None, :] // 128) * 16 + (p[:, None] % 16))).astype(np.float32)
    sel_lhsT = np.zeros((128, 256), dtype=np.float32)
    selmask = np.zeros((128, 64), dtype=np.float32)
    for chunk in range(2):
        c = chunk * 128 + np.arange(128)
        sel_lhsT[:, chunk * 128 : chunk * 128 + 128] = (
            (c[:, None] % 8) == (p[None, :] // 16)
        ).astype(np.float32)
        selmask[:, chunk * 32 : chunk * 32 + 32] = (
            (c[:, None] // 8) == np.arange(32)[None, :]
        ).astype(np.float32)
    return dict(iww=iww, ihw=ihw, iwp=iwp, ihp=ihp,
                sel_lhsT=sel_lhsT, selmask=selmask)


def _col2(x):
    return np.ascontiguousarray(x.reshape(2, 128).T)


def _mm_layout(Wm, n_out):
    return np.ascontiguousarray(
        Wm.reshape(2, 128, n_out).transpose(1, 0, 2).reshape(128, 2 * n_out)
    )


def _build(trace_label=""):
    import concourse.bass as bass
    import concourse.tile as tile
    from concourse import bacc, mybir
    from concourse.bass import ds

    f32, i32, i16 = mybir.dt.float32, mybir.dt.int32, mybir.dt.int16
    bf16 = mybir.dt.bfloat16
    AF = mybir.ActivationFunctionType
    OP = mybir.AluOpType

    nc = bacc.Bacc("TRN2", target_bir_lowering=False, debug=False,
                   num_devices=NCORES)

    def din(name, shape):
        return nc.dram_tensor(name, list(shape), f32, kind="ExternalInput").ap()

    fmb = din("fmb", [D * CH_PITCHF])
    pc = din("pc", [128, 2])
    w1 = din("w1", [128, 512])
    ws = din("ws", [128, 512])
    wr = din("wr", [128, 512])
    wt = din("wt", [128, 1024])
    b1 = din("b1", [128, 2])
    bs = din("bs", [128, 2])
    br = din("br", [128, 2])
    bt = din("bt", [128, 4])
    iww_d = din("iww", [128, 512])
    ihw_d = din("ihw", [128, 512])
    iwp_d = din("iwp", [128, 512])
    ihp_d = din("ihp", [128, 512])
    sel_lhsT_d = din("sel_lhsT", [128, 256])
    selmask_d = din("selmask", [128, 64])
    out_d = nc.dram_tensor("out", [D * NPIX], bf16,
                           kind="ExternalOutput").ap()

    fmb_b = fmb.bitcast(bf16).rearrange("(G q) -> G q", G=D)  # [256, 16896]
    out5 = out_d.rearrange("(G c r x w) -> G c r x w",
                           G=D, c=4, r=16, x=2, w=128)

    with tile.TileContext(nc) as tc:
        with (
            tc.tile_pool(name="persist", bufs=1) as persist,
            tc.tile_pool(name="psum", bufs=1, space="PSUM") as psum,
            tc.tile_pool(name="src", bufs=2) as srcp,
            tc.tile_pool(name="gath", bufs=2) as gathp,
            tc.tile_pool(name="bi16", bufs=2) as bi16p,
            tc.tile_pool(name="prc", bufs=2) as prcp,
            tc.tile_pool(name="outp", bufs=2) as outp,
            tc.tile_pool(name="ws", bufs=1) as wsp,
            tc.tile_pool(name="widx", bufs=1) as widxp,
        ):
            # ---- staging ----
            def stage_p(ap_dram, shape, tag):
                t = persist.tile(list(shape), f32, tag=tag)
                nc.sync.dma_start(t[:], ap_dram[:])
                return t

            pc_t = stage_p(pc, [128, 2], "s_pc")
            b1_t = stage_p(b1, [128, 2], "s_b1")
            bs_t = stage_p(bs, [128, 2], "s_bs")
            br_t = stage_p(br, [128, 2], "s_br")
            bt_t = stage_p(bt, [128, 4], "s_bt")
            selmask_t = stage_p(selmask_d, [128, 64], "s_selmask")
            # bf16 iotas (values 0..254, exact); gpsimd DMA casts f32->bf16
            iww_t = persist.tile([128, 512], bf16, tag="s_iww")
            nc.gpsimd.dma_start(iww_t[:], iww_d[:])
            ihw_t = persist.tile([128, 512], bf16, tag="s_ihw")
            nc.gpsimd.dma_start(ihw_t[:], ihw_d[:])
            iwp_t = persist.tile([128, 512], bf16, tag="s_iwp")
            nc.gpsimd.dma_start(iwp_t[:], iwp_d[:])
            ihp_t = persist.tile([128, 512], bf16, tag="s_ihp")
            nc.gpsimd.dma_start(ihp_t[:], ihp_d[:])
            # transient weights -> main-loop tags (dead after setup)
            w1_t = gathp.tile([128, 512], f32, tag="G")
            nc.sync.dma_start(w1_t[:], w1[:])
            ws_t = widxp.tile([128, 512], f32, tag="wa")
            nc.sync.dma_start(ws_t[:], ws[:])
            wr_t = widxp.tile([128, 512], f32, tag="wb")
            nc.sync.dma_start(wr_t[:], wr[:])
            wt_t = srcp.tile([128, 1024], f32, tag="S")
            nc.sync.dma_start(wt_t[:], wt[:])
            sel_lhsT_t = widxp.tile([128, 256], f32, tag="wc")
            nc.sync.dma_start(sel_lhsT_t[:], sel_lhsT_d[:])

            # ---- param MLP in column layout ----
            p_sb = persist.tile([128, 2], f32)

            def mlp_cols(w_tile, rhs_tile, bias_tile, n_chunks_out, func,
                         out_tile, scale=1.0, n_out_cols=256):
                for m in range(n_chunks_out):
                    ps = psum.tile([128, 1], f32, space="PSUM")
                    for kk in range(2):
                        nc.tensor.matmul(
                            ps[:],
                            lhsT=w_tile[:, kk * n_out_cols + m * 128 :
                                        kk * n_out_cols + m * 128 + 128],
                            rhs=rhs_tile[:, kk : kk + 1],
                            start=(kk == 0), stop=(kk == 1),
                        )
                    nc.scalar.activation(out_tile[:, m : m + 1], ps[:], func,
                                         bias=bias_tile[:, m : m + 1],
                                         scale=scale)

            mlp_cols(w1_t, pc_t, b1_t, 2, AF.Relu, p_sb)
            sig_sb = persist.tile([128, 2], f32)
            mlp_cols(ws_t, p_sb, bs_t, 2, AF.Sigmoid, sig_sb)
            tnh_sb = persist.tile([128, 2], f32)
            mlp_cols(wr_t, p_sb, br_t, 2, AF.Tanh, tnh_sb)
            tt_sb = persist.tile([128, 4], f32)
            mlp_cols(wt_t, p_sb, bt_t, 4, AF.Tanh, tt_sb, n_out_cols=512)

            zero_b = persist.tile([128, 1], f32)
            nc.vector.memset(zero_b[:], 0.0)
            cs_sb = persist.tile([128, 2], f32)
            sn_sb = persist.tile([128, 2], f32)
            sh_sb = persist.tile([128, 2], f32)
            for m in range(2):
                # sin LUT is only accurate on ~[-pi, pi]; cos via half-angle
                nc.scalar.activation(sn_sb[:, m : m + 1], tnh_sb[:, m : m + 1],
                                     AF.Sin, bias=zero_b[:], scale=PI)
                nc.scalar.activation(sh_sb[:, m : m + 1], tnh_sb[:, m : m + 1],
                                     AF.Sin, bias=zero_b[:], scale=PI / 2.0)
                nc.scalar.activation(sh_sb[:, m : m + 1], sh_sb[:, m : m + 1],
                                     AF.Square, bias=zero_b[:], scale=1.0)
                nc.vector.tensor_scalar(cs_sb[:, m : m + 1],
                                        sh_sb[:, m : m + 1], -2.0, 1.0,
                                        op0=OP.mult, op1=OP.add)

            # ---- affine coefficients (pixel space) ----
            P10 = persist.tile([128, 10], f32)
            AXF = 256.0 / 127.0
            for m in range(2):
                o = m * 5
                csig = persist.tile([128, 1], f32, tag="csig")
                ssig = persist.tile([128, 1], f32, tag="ssig")
                nc.vector.tensor_tensor(csig[:], cs_sb[:, m : m + 1],
                                        sig_sb[:, m : m + 1], op=OP.mult)
                nc.vector.tensor_tensor(ssig[:], sn_sb[:, m : m + 1],
                                        sig_sb[:, m : m + 1], op=OP.mult)
                nc.vector.tensor_scalar(P10[:, o + 2 : o + 3], csig[:], AXF,
                                        None, op0=OP.mult)
                nc.vector.tensor_scalar(P10[:, o : o + 1], ssig[:], -AXF,
                                        None, op0=OP.mult)
                nc.vector.tensor_scalar(P10[:, o + 4 : o + 5], ssig[:], AXF,
                                        None, op0=OP.mult)  # bxn = -bx
                e1 = persist.tile([128, 1], f32, tag="e1")
                nc.vector.tensor_scalar(e1[:], tt_sb[:, m : m + 1], 64.0, 63.5,
                                        op0=OP.mult, op1=OP.add)
                e2 = persist.tile([128, 1], f32, tag="e2")
                nc.vector.scalar_tensor_tensor(e2[:], csig[:], -128.0, e1[:],
                                               op0=OP.mult, op1=OP.add)
                nc.vector.scalar_tensor_tensor(P10[:, o + 1 : o + 2], ssig[:],
                                               128.0, e2[:],
                                               op0=OP.mult, op1=OP.add)
                f1 = persist.tile([128, 1], f32, tag="f1")
                nc.vector.tensor_scalar(f1[:], tt_sb[:, m + 2 : m + 3], 64.0,
                                        63.5, op0=OP.mult, op1=OP.add)
                f2 = persist.tile([128, 1], f32, tag="f2")
                nc.vector.scalar_tensor_tensor(f2[:], ssig[:], -128.0, f1[:],
                                               op0=OP.mult, op1=OP.add)
                nc.vector.scalar_tensor_tensor(P10[:, o + 3 : o + 4], csig[:],
                                               -128.0, f2[:],
                                               op0=OP.mult, op1=OP.add)

            # ---- grouped coefficient tables [128, 32] ----
            grp = {}
            for name, t_idx in (("ax", 2), ("bx", 0), ("ex", 1), ("ey", 3),
                                ("bxn", 4)):
                g_ps = psum.tile([128, 32], f32, space="PSUM")
                for m in range(2):
                    rhs = persist.tile([128, 32], f32, tag="grp_rhs")
                    nc.vector.tensor_scalar(
                        rhs[:], selmask_t[:, m * 32 : m * 32 + 32],
                        P10[:, m * 5 + t_idx : m * 5 + t_idx + 1], None,
                        op0=OP.mult)
                    nc.tensor.matmul(g_ps[:],
                                     lhsT=sel_lhsT_t[:, m * 128 : m * 128 + 128],
                                     rhs=rhs[:], start=(m == 0), stop=(m == 1))
                g_sb = persist.tile([128, 32], f32, tag=f"grp_{name}")
                nc.vector.tensor_copy(g_sb[:], g_ps[:])
                grp[name] = g_sb

            # ---- per-pair anchor gather index (wrapped layout) for call j ----
            def compute_idx(j):
                kk = ds(j, 1)
                ax_c, bx_c = grp["ax"][:, kk], grp["bx"][:, kk]
                bxn_c, ex_c = grp["bxn"][:, kk], grp["ex"][:, kk]
                ey_c = grp["ey"][:, kk]

                def coord_floors(iw_t, ih_t, sa, sb, se, te_tag, to_tag):
                    te = widxp.tile([128, 512], f32, tag=te_tag, name="te")
                    to = widxp.tile([128, 512], f32, tag=to_tag, name="to")
                    fr = widxp.tile([128, 512], f32, tag="wsc", name="fr")
                    nc.vector.tensor_scalar(te[:], iw_t[:], sa, None,
                                            op0=OP.mult)
                    nc.vector.scalar_tensor_tensor(te[:], ih_t[:], sb, te[:],
                                                   op0=OP.mult, op1=OP.add)
                    nc.vector.tensor_scalar(te[:], te[:], se, None, op0=OP.add)
                    nc.vector.tensor_scalar(to[:], te[:], sb, None, op0=OP.add)
                    for q in (te, to):
                        nc.vector.tensor_scalar(q[:], q[:], 0.0, 127.0,
                                                op0=OP.max, op1=OP.min)
                        # floor via round-magic: fr=round(q); fr=q-fr (+1 if
                        # negative) = frac; q -= frac
                        nc.vector.tensor_scalar(fr[:], q[:], 12582912.0,
                                                -12582912.0,
                                                op0=OP.add, op1=OP.add)
                        nc.vector.tensor_tensor(fr[:], q[:], fr[:],
                                                op=OP.subtract)
                        nc.vector.scalar_tensor_tensor(fr[:], fr[:], 0.0,
                                                       fr[:], op0=OP.is_lt,
                                                       op1=OP.add)
                        nc.vector.tensor_tensor(q[:], q[:], fr[:],
                                                op=OP.subtract)
                    return te, to

                x0e, x0o = coord_floors(iww_t, ihw_t, ax_c, bx_c, ex_c,
                                        "wa", "wb")
                nc.vector.tensor_tensor(x0e[:], x0e[:], x0o[:], op=OP.min)
                # fl = floor(axm/2); block index jb = aym*64 + fl
                fl = widxp.tile([128, 512], f32, tag="wb", name="fl")
                fr2 = widxp.tile([128, 512], f32, tag="wsc", name="fr2")
                nc.vector.tensor_scalar(fl[:], x0e[:], 0.5, None, op0=OP.mult)
                nc.vector.tensor_scalar(fr2[:], fl[:], 12582912.0, -12582912.0,
                                        op0=OP.add, op1=OP.add)
                nc.vector.tensor_tensor(fr2[:], fl[:], fr2[:], op=OP.subtract)
                nc.vector.scalar_tensor_tensor(fr2[:], fr2[:], 0.0, fr2[:],
                                               op0=OP.is_lt, op1=OP.add)
                nc.vector.tensor_tensor(fl[:], fl[:], fr2[:], op=OP.subtract)
                y0e, y0o = coord_floors(iww_t, ihw_t, bxn_c, ax_c, ey_c,
                                        "wa", "wc")
                nc.vector.tensor_tensor(y0e[:], y0e[:], y0o[:], op=OP.min)
                nc.vector.scalar_tensor_tensor(fl[:], y0e[:], 64.0, fl[:],
                                               op0=OP.mult, op1=OP.add)
                b32 = widxp.tile([128, 512], i32, tag="wc", name="b32")
                nc.vector.tensor_copy(b32[:], fl[:])
                bt_ = bi16p.tile([128, 512], i16, tag="bi16")
                nc.vector.tensor_copy(bt_[:], b32[:])
                return bt_

            bi16_tiles = [None] * (CALLS + 1)
            bi16_tiles[0] = compute_idx(0)

            x_engs = [nc.scalar, nc.sync]

            # 16-lane bf16-singles source load for call j; the first 11 lanes
            # ride the SWDGE (gpsimd) queue -- dispatched one call AHEAD so
            # their Pool-queue slot lands between gather streams.
            def load_S(j):
                S = srcp.tile([128, NPIX // 2], f32, tag="S", name="S")
                Sb = S[:].bitcast(bf16)
                for li, (r, cx) in enumerate(LANES):
                    sig = 128 * r + cx
                    eng = (nc.gpsimd if li < 12
                           else (nc.sync if li < 14 else nc.scalar))
                    eng.dma_start(
                        Sb[li::16, :],
                        fmb_b[ds(8 * j, 8), ds(sig, NPIX)],
                    )
                return S

            S_tiles = [None] * CALLS
            S_tiles[0] = load_S(0)

            # ================= main loop =================
            for k in range(CALLS):
                kk = ds(k, 1)
                if k + 1 < CALLS:
                    S_tiles[k + 1] = load_S(k + 1)

                # ---- indices for the NEXT call (gather k+1 never waits) ----
                if k + 1 < CALLS:
                    bi16_tiles[k + 1] = compute_idx(k + 1)
                S = S_tiles[k]

                bi16 = bi16_tiles[k]
                ax_c, bx_c = grp["ax"][:, kk], grp["bx"][:, kk]
                bxn_c, ex_c = grp["bxn"][:, kk], grp["ex"][:, kk]
                ey_c = grp["ey"][:, kk]

                for sc in range(SC):
                    P = [prcp.tile([128, SCW], f32, tag=f"P{l}",
                                   name=f"P{l}")
                         for l in range(16)]
                    # ---- gather 2 chunks + lane extraction ----
                    for cc in range(2):
                        c = sc * 2 + cc
                        G = gathp.tile([128, CHUNK], f32, tag="G")
                        nc.gpsimd.ap_gather(
                            G[:].bitcast(bf16), S[:].bitcast(bf16),
                            bi16[:, ds(c * 128, 128)],
                            channels=128, num_elems=NPIX // 2, d=2,
                            num_idxs=CHUNK)
                        for li in range(16):
                            x_engs[li % 2].dma_start(
                                P[li][:, ds(cc * 128, 128)],
                                G[:][li::16, :]
                                .rearrange("g (r w) -> g r w", r=16),
                            )

                    # ---- per-pixel coords/fracs/offsets in P_rc layout ----
                    ss = ds(sc * SCW, SCW)
                    iwp_s, ihp_s = iwp_t[:, ss], ihp_t[:, ss]

                    def coord_full(sa, sb, se, tags):
                        te = wsp.tile([128, SCW], f32, tag=tags[0])
                        to = wsp.tile([128, SCW], f32, tag=tags[1])
                        fe = wsp.tile([128, SCW], f32, tag=tags[2])
                        fo = wsp.tile([128, SCW], f32, tag=tags[3])
                        nc.vector.tensor_scalar(te[:], iwp_s, sa, None,
                                                op0=OP.mult)
                        nc.vector.scalar_tensor_tensor(te[:], ihp_s, sb, te[:],
                                                       op0=OP.mult, op1=OP.add)
                        nc.vector.tensor_scalar(te[:], te[:], se, None,
                                                op0=OP.add)
                        nc.vector.tensor_scalar(to[:], te[:], sb, None,
                                                op0=OP.add)
                        for q, fq in ((te, fe), (to, fo)):
                            nc.vector.tensor_scalar(q[:], q[:], 0.0, 127.0,
                                                    op0=OP.max, op1=OP.min)
                            nc.vector.tensor_scalar(fq[:], q[:], 12582912.0,
                                                    -12582912.0,
                                                    op0=OP.add, op1=OP.add)
                            nc.vector.tensor_tensor(fq[:], q[:], fq[:],
                                                    op=OP.subtract)
                            nc.vector.scalar_tensor_tensor(fq[:], fq[:], 0.0,
                                                           fq[:],
                                                           op0=OP.is_lt,
                                                           op1=OP.add)
                            nc.vector.tensor_tensor(q[:], q[:], fq[:],
                                                    op=OP.subtract)
                        return te, to, fe, fo

                    x0e, x0o, fxe, fxo = coord_full(
                        ax_c, bx_c, ex_c, ("px0", "px1", "pfx0", "pfx1"))
                    y0e, y0o, fye, fyo = coord_full(
                        bxn_c, ax_c, ey_c, ("py0", "py1", "pfy0", "pfy1"))
                    axm = wsp.tile([128, SCW], f32, tag="paxm")
                    aym = wsp.tile([128, SCW], f32, tag="paym")
                    nc.vector.tensor_tensor(axm[:], x0e[:], x0o[:], op=OP.min)
                    nc.vector.tensor_tensor(aym[:], y0e[:], y0o[:], op=OP.min)
                    # par = axm mod 2 (block-misalignment of the d=2 gather)
                    par = wsp.tile([128, SCW], f32, tag="ppar")
                    pt = wsp.tile([128, SCW], f32, tag="ppt")
                    nc.vector.tensor_scalar(pt[:], axm[:], 0.5, None,
                                            op0=OP.mult)
                    nc.vector.tensor_scalar(par[:], pt[:], 12582912.0,
                                            -12582912.0, op0=OP.add,
                                            op1=OP.add)
                    nc.vector.tensor_tensor(par[:], pt[:], par[:],
                                            op=OP.subtract)
                    nc.vector.scalar_tensor_tensor(par[:], par[:], 0.0,
                                                   par[:], op0=OP.is_lt,
                                                   op1=OP.add)
                    nc.vector.tensor_scalar(par[:], par[:], 2.0, None,
                                            op0=OP.mult)
                    # deltas: dx in 0..2 (+par -> 0..3), dy in 0..2
                    dups = []
                    for dn, (pos, anc, addpar) in enumerate(
                            ((x0e, axm, True), (x0o, axm, True),
                             (y0e, aym, False), (y0o, aym, False))):
                        nc.vector.tensor_tensor(pos[:], pos[:], anc[:],
                                                op=OP.subtract)
                        if addpar:
                            nc.vector.tensor_tensor(pos[:], pos[:], par[:],
                                                    op=OP.add)
                        dd = wsp.tile([128, 2 * SCW], bf16, tag=f"dd{dn}")
                        nc.vector.tensor_copy(dd[:, 0::2], pos[:])
                        nc.vector.tensor_copy(dd[:, 1::2], pos[:])
                        dups.append(dd)
                    dxe, dxo, dye, dyo = dups

                    Lb = [P[l][:].bitcast(bf16) for l in range(16)]

                    # ---- masked patch selection + bilinear, per class ----
                    for cls, (dxd, dyd, fx, fy, par) in enumerate(
                            ((dxe, dye, fxe, fye, 0),
                             (dxo, dyo, fxo, fyo, 1))):
                        Cr = []
                        Ct = wsp.tile([128, 2 * SCW], bf16, tag="paxm",
                                      name="Ct")
                        ctags = ("px0", "px1", "py0", "py1")
                        for r in range(4):
                            C = wsp.tile([128, 2 * SCW], bf16, tag=ctags[r],
                                         name=f"C{r}")
                            nc.vector.scalar_tensor_tensor(
                                C[:], dxd[:], 0.0, Lb[r * 4 + 0],
                                op0=OP.is_equal, op1=OP.mult)
                            for cx in (1, 2, 3):
                                nc.vector.scalar_tensor_tensor(
                                    Ct[:], dxd[:], float(cx), Lb[r * 4 + cx],
                                    op0=OP.is_equal, op1=OP.mult)
                                nc.vector.tensor_tensor(C[:], C[:], Ct[:],
                                                        op=OP.add)
                            Cr.append(C)
                        PK = []
                        ktags = ("paym", "K1")
                        for tap in range(2):
                            Kt = wsp.tile([128, 2 * SCW], bf16,
                                          tag=ktags[tap], name=f"K{tap}")
                            nc.vector.scalar_tensor_tensor(
                                Kt[:], dyd[:], 0.0, Cr[tap][:],
                                op0=OP.is_equal, op1=OP.mult)
                            for rr in (1, 2):
                                nc.vector.scalar_tensor_tensor(
                                    Ct[:], dyd[:], float(rr), Cr[tap + rr][:],
                                    op0=OP.is_equal, op1=OP.mult)
                                nc.vector.tensor_tensor(Kt[:], Kt[:], Ct[:],
                                                        op=OP.add)
                            PK.append(Kt)
                        # x-lerp from packed bf16 pairs, then y-lerp
                        Rs = []
                        for tap in range(2):
                            lo = PK[tap][:, 0::2]
                            hi = PK[tap][:, 1::2]
                            Rt = wsp.tile([128, SCW], f32, tag=f"R{tap}")
                            nc.vector.tensor_tensor(Rt[:], hi, lo,
                                                    op=OP.subtract)
                            nc.vector.tensor_tensor(Rt[:], Rt[:], fx[:],
                                                    op=OP.mult)
                            nc.vector.tensor_tensor(Rt[:], Rt[:], lo,
                                                    op=OP.add)
                            Rs.append(Rt)
                        O = outp.tile([128, SCW], bf16, tag=f"O{cls}")
                        nc.vector.tensor_tensor(Rs[1][:], Rs[1][:], Rs[0][:],
                                                op=OP.subtract)
                        nc.vector.tensor_tensor(Rs[1][:], Rs[1][:], fy[:],
                                                op=OP.mult)
                        nc.vector.tensor_tensor(O[:], Rs[1][:], Rs[0][:],
                                                op=OP.add)
                        # ---- output DMA: rows h = 2v+par, contiguous w ----
                        for cc in range(2):
                            x_engs[(cls + cc) % 2].dma_start(
                                out5[ds(8 * k, 8), sc * 2 + cc,
                                     slice(None), par, slice(None)],
                                O[:, ds(cc * 128, 128)],
                            )

    nc.compile()
    return nc


def _prepare_in_maps(feature_map, para_code, W1, b1, Ws, bs, Wr, br, Wt, bt):
    import ml_dtypes

    consts = _host_constants()
    Wt_re = np.concatenate([Wt[:, 0::2], Wt[:, 1::2]], axis=1)
    bt_re = np.concatenate([bt[0::2], bt[1::2]])
    common = dict(
        w1=_mm_layout(W1, 256), ws=_mm_layout(Ws, 256), wr=_mm_layout(Wr, 256),
        wt=_mm_layout(Wt_re, 512),
        b1=_col2(b1), bs=_col2(bs), br=_col2(br),
        bt=np.ascontiguousarray(bt_re.reshape(4, 128).T),
        **consts,
    )
    common = {k: np.ascontiguousarray(v, dtype=np.float32)
              for k, v in common.items()}
    in_maps = []
    for i in range(NCORES):
        flat = np.ascontiguousarray(feature_map[i].reshape(D, NPIX),
                                    dtype=np.float32)
        fmb = np.zeros((D, CH_PITCHB), dtype=ml_dtypes.bfloat16)
        fmb[:, :NPIX] = flat.astype(ml_dtypes.bfloat16)
        m = dict(common)
        m["fmb"] = np.ascontiguousarray(fmb).view(np.float32).reshape(-1)
        m["pc"] = _col2(para_code[i])
        in_maps.append(m)
    return in_maps


def _run(inputs, trace=False):
    from concourse.bass_utils import run_bass_kernel_spmd

    if "nc" not in _GRAPH_CACHE:
        _GRAPH_CACHE["nc"] = _build()
    nc = _GRAPH_CACHE["nc"]
    in_maps = _prepare_in_maps(**inputs)
    res = run_bass_kernel_spmd(nc, in_maps, core_ids=list(range(NCORES)),
                               trace=trace)
    out = np.stack([
        np.asarray(res.results[i]["out"]).astype(np.float32).reshape(D, H, W)
        for i in range(NCORES)
    ])
    return out, res


def kernel(**inputs) -> np.ndarray:
    out, _ = _run(inputs, trace=False)
    return out


# revision 27
# speedup vs baseline: 1.2246x; 1.0342x over previous
"""AdaAT (per-channel affine warp + bilinear grid_sample) on 8 TRN2 NeuronCores.

Sharding: data-parallel over batch (B=8 -> 1 sample per core).

v5: the ap_gather ucode wall (~27ns/index-slot) is halved by PAIR-SHARING:
vertically-adjacent output pixels (2v, w) and (2v+1, w) share ONE gather
index (the elementwise-min "anchor" of their floored source coords).  Each
gathered 4-byte lane value is a bf16-packed horizontal pair
(bf16 s[j], bf16 s[j+1]), so one lane covers both x-taps of a pixel.  The
16 lanes of each channel group hold the packed source image shifted by
128*r + c for (r, c) in 4 rows x 3 cols -- a patch that covers every
(row-tap, x-pair) offset either pixel of the pair can need, since the
per-pixel affine steps satisfy |ax|,|bx| <= 2 (data max ~1.58; holds for
any scale <= 1.984).  Per-pixel offsets into the patch are resolved
post-gather on the DVE with is_equal-masked sums over the 12 lanes
(bf16 ops, exact for {0,1}-masks), then bilinear-combined with f32 fracs.

Index slots per core: 32 calls x 8192 pairs = 262144 (vs 524288) ->
gather ~7.1ms.  All DMA (12-lane source replication ~6.2MB/call, lane
extraction ~3MB/call, output) is spread across the sync/scalar/tensor/
vector HWDGE rings to hide under the gather stream.
"""

import numpy as np

B, D, H, W = 8, 256, 128, 128
NPIX = H * W  # 16384
NCORES = 8
PI = 3.14159  # matches reference
CALLS = D // 8  # 32 calls per core, 8 channels each
NPAIR = NPIX // 2  # 8192 pair-slots per channel per call
NCHUNK = 4
CHUNK = NPAIR // NCHUNK  # 2048 slots per ap_gather
SC = 2  # DVE super-chunks per call (2 gather chunks each)
SCW = NPAIR // SC // 16  # 256: free width of per-sc tiles
PAD = 512
CH_PITCHB = NPIX + PAD      # per-channel pitch in bf16 elements
CH_PITCHF = CH_PITCHB // 2  # ... in f32 words
LANES = [(r, c) for r in range(4) for c in range(4)]  # all 16 lanes

_GRAPH_CACHE = {}


def _host_constants():
    p = np.arange(128)
    s = np.arange(512)
    # wrapped-16 pair iotas: pair slot q = s*16 + p%16; v = q//128, w = q%128
    q = s[None, :] * 16 + (p[:, None] % 16)  # [128, 512]
    iww = (q % 128).astype(np.float32)
    ihw = (2 * (q // 128)).astype(np.float32)
    # P_rc-layout iotas: partition 16g+r, f: v = (f//128)*16 + r, w = f%128
    f = np.arange(512)
    iwp = np.broadcast_to((f % 128).astype(np.float32), (128, 512)).copy()
    ihp = (2 * ((f[None, :] // 128) * 16 + (p[:, None] % 16))).astype(np.float32)
    sel_lhsT = np.zeros((128, 256), dtype=np.float32)
    selmask = np.zeros((128, 64), dtype=np.float32)
    for chunk in range(2):
        c = chunk * 128 + np.arange(128)
        sel_lhsT[:, chunk * 128 : chunk * 128 + 128] = (
            (c[:, None] % 8) == (p[None, :] // 16)
        ).astype(np.float32)
        selmask[:, chunk * 32 : chunk * 32 + 32] = (
            (c[:, None] // 8) == np.arange(32)[None, :]
        ).astype(np.float32)
    return dict(iww=iww, ihw=ihw, iwp=iwp, ihp=ihp,
                sel_lhsT=sel_lhsT, selmask=selmask)


def _col2(x):
    return np.ascontiguousarray(x.reshape(2, 128).T)


def _mm_layout(Wm, n_out):
    return np.ascontiguousarray(
        Wm.reshape(2, 128, n_out).transpose(1, 0, 2).reshape(128, 2 * n_out)
    )


def _build(trace_label=""):
    import concourse.bass as bass
    import concourse.tile as tile
    from concourse import bacc, mybir
    from concourse.bass import ds

    f32, i32, i16 = mybir.dt.float32, mybir.dt.int32, mybir.dt.int16
    bf16 = mybir.dt.bfloat16
    AF = mybir.ActivationFunctionType
    OP = mybir.AluOpType

    nc = bacc.Bacc("TRN2", target_bir_lowering=False, debug=False,
                   num_devices=NCORES)

    def din(name, shape):
        return nc.dram_tensor(name, list(shape), f32, kind="ExternalInput").ap()

    fmb = din("fmb", [D * CH_PITCHF])
    pc = din("pc", [128, 2])
    w1 = din("w1", [128, 512])
    ws = din("ws", [128, 512])
    wr = din("wr", [128, 512])
    wt = din("wt", [128, 1024])
    b1 = din("b1", [128, 2])
    bs = din("bs", [128, 2])
    br = din("br", [128, 2])
    bt = din("bt", [128, 4])
    iww_d = din("iww", [128, 512])
    ihw_d = din("ihw", [128, 512])
    iwp_d = din("iwp", [128, 512])
    ihp_d = din("ihp", [128, 512])
    sel_lhsT_d = din("sel_lhsT", [128, 256])
    selmask_d = din("selmask", [128, 64])
    out_d = nc.dram_tensor("out", [D * NPIX], bf16,
                           kind="ExternalOutput").ap()

    fmb_b = fmb.bitcast(bf16).rearrange("(G q) -> G q", G=D)  # [256, 16896]
    out5 = out_d.rearrange("(G c r x w) -> G c r x w",
                           G=D, c=4, r=16, x=2, w=128)

    with tile.TileContext(nc) as tc:
        with (
            tc.tile_pool(name="persist", bufs=1) as persist,
            tc.tile_pool(name="psum", bufs=1, space="PSUM") as psum,
            tc.tile_pool(name="src", bufs=2) as srcp,
            tc.tile_pool(name="gath", bufs=2) as gathp,
            tc.tile_pool(name="bi16", bufs=2) as bi16p,
            tc.tile_pool(name="prc", bufs=2) as prcp,
            tc.tile_pool(name="outp", bufs=2) as outp,
            tc.tile_pool(name="ws", bufs=1) as wsp,
            tc.tile_pool(name="widx", bufs=1) as widxp,
        ):
            # ---- staging ----
            def stage_p(ap_dram, shape, tag):
                t = persist.tile(list(shape), f32, tag=tag)
                nc.sync.dma_start(t[:], ap_dram[:])
                return t

            pc_t = stage_p(pc, [128, 2], "s_pc")
            b1_t = stage_p(b1, [128, 2], "s_b1")
            bs_t = stage_p(bs, [128, 2], "s_bs")
            br_t = stage_p(br, [128, 2], "s_br")
            bt_t = stage_p(bt, [128, 4], "s_bt")
            selmask_t = stage_p(selmask_d, [128, 64], "s_selmask")
            # bf16 iotas (values 0..254, exact); gpsimd DMA casts f32->bf16
            iww_t = persist.tile([128, 512], bf16, tag="s_iww")
            nc.gpsimd.dma_start(iww_t[:], iww_d[:])
            ihw_t = persist.tile([128, 512], bf16, tag="s_ihw")
            nc.gpsimd.dma_start(ihw_t[:], ihw_d[:])
            iwp_t = persist.tile([128, 512], bf16, tag="s_iwp")
            nc.gpsimd.dma_start(iwp_t[:], iwp_d[:])
            ihp_t = persist.tile([128, 512], bf16, tag="s_ihp")
            nc.gpsimd.dma_start(ihp_t[:], ihp_d[:])
            # transient weights -> main-loop tags (dead after setup)
            w1_t = gathp.tile([128, 512], f32, tag="G")
            nc.sync.dma_start(w1_t[:], w1[:])
            ws_t = widxp.tile([128, 512], f32, tag="wa")
            nc.sync.dma_start(ws_t[:], ws[:])
            wr_t = widxp.tile([128, 512], f32, tag="wb")
            nc.sync.dma_start(wr_t[:], wr[:])
            wt_t = srcp.tile([128, 1024], f32, tag="S")
            nc.sync.dma_start(wt_t[:], wt[:])
            sel_lhsT_t = widxp.tile([128, 256], f32, tag="wc")
            nc.sync.dma_start(sel_lhsT_t[:], sel_lhsT_d[:])

            # ---- param MLP in column layout ----
            p_sb = persist.tile([128, 2], f32)

            def mlp_cols(w_tile, rhs_tile, bias_tile, n_chunks_out, func,
                         out_tile, scale=1.0, n_out_cols=256):
                for m in range(n_chunks_out):
                    ps = psum.tile([128, 1], f32, space="PSUM")
                    for kk in range(2):
                        nc.tensor.matmul(
                            ps[:],
                            lhsT=w_tile[:, kk * n_out_cols + m * 128 :
                                        kk * n_out_cols + m * 128 + 128],
                            rhs=rhs_tile[:, kk : kk + 1],
                            start=(kk == 0), stop=(kk == 1),
                        )
                    nc.scalar.activation(out_tile[:, m : m + 1], ps[:], func,
                                         bias=bias_tile[:, m : m + 1],
                                         scale=scale)

            mlp_cols(w1_t, pc_t, b1_t, 2, AF.Relu, p_sb)
            sig_sb = persist.tile([128, 2], f32)
            mlp_cols(ws_t, p_sb, bs_t, 2, AF.Sigmoid, sig_sb)
            tnh_sb = persist.tile([128, 2], f32)
            mlp_cols(wr_t, p_sb, br_t, 2, AF.Tanh, tnh_sb)
            tt_sb = persist.tile([128, 4], f32)
            mlp_cols(wt_t, p_sb, bt_t, 4, AF.Tanh, tt_sb, n_out_cols=512)

            zero_b = persist.tile([128, 1], f32)
            nc.vector.memset(zero_b[:], 0.0)
            cs_sb = persist.tile([128, 2], f32)
            sn_sb = persist.tile([128, 2], f32)
            sh_sb = persist.tile([128, 2], f32)
            for m in range(2):
                # sin LUT is only accurate on ~[-pi, pi]; cos via half-angle
                nc.scalar.activation(sn_sb[:, m : m + 1], tnh_sb[:, m : m + 1],
                                     AF.Sin, bias=zero_b[:], scale=PI)
                nc.scalar.activation(sh_sb[:, m : m + 1], tnh_sb[:, m : m + 1],
                                     AF.Sin, bias=zero_b[:], scale=PI / 2.0)
                nc.scalar.activation(sh_sb[:, m : m + 1], sh_sb[:, m : m + 1],
                                     AF.Square, bias=zero_b[:], scale=1.0)
                nc.vector.tensor_scalar(cs_sb[:, m : m + 1],
                                        sh_sb[:, m : m + 1], -2.0, 1.0,
                                        op0=OP.mult, op1=OP.add)

            # ---- affine coefficients (pixel space) ----
            P10 = persist.tile([128, 10], f32)
            AXF = 256.0 / 127.0
            for m in range(2):
                o = m * 5
                csig = persist.tile([128, 1], f32, tag="csig")
                ssig = persist.tile([128, 1], f32, tag="ssig")
                nc.vector.tensor_tensor(csig[:], cs_sb[:, m : m + 1],
                                        sig_sb[:, m : m + 1], op=OP.mult)
                nc.vector.tensor_tensor(ssig[:], sn_sb[:, m : m + 1],
                                        sig_sb[:, m : m + 1], op=OP.mult)
                nc.vector.tensor_scalar(P10[:, o + 2 : o + 3], csig[:], AXF,
                                        None, op0=OP.mult)
                nc.vector.tensor_scalar(P10[:, o : o + 1], ssig[:], -AXF,
                                        None, op0=OP.mult)
                nc.vector.tensor_scalar(P10[:, o + 4 : o + 5], ssig[:], AXF,
                                        None, op0=OP.mult)  # bxn = -bx
                e1 = persist.tile([128, 1], f32, tag="e1")
                nc.vector.tensor_scalar(e1[:], tt_sb[:, m : m + 1], 64.0, 63.5,
                                        op0=OP.mult, op1=OP.add)
                e2 = persist.tile([128, 1], f32, tag="e2")
                nc.vector.scalar_tensor_tensor(e2[:], csig[:], -128.0, e1[:],
                                               op0=OP.mult, op1=OP.add)
                nc.vector.scalar_tensor_tensor(P10[:, o + 1 : o + 2], ssig[:],
                                               128.0, e2[:],
                                               op0=OP.mult, op1=OP.add)
                f1 = persist.tile([128, 1], f32, tag="f1")
                nc.vector.tensor_scalar(f1[:], tt_sb[:, m + 2 : m + 3], 64.0,
                                        63.5, op0=OP.mult, op1=OP.add)
                f2 = persist.tile([128, 1], f32, tag="f2")
                nc.vector.scalar_tensor_tensor(f2[:], ssig[:], -128.0, f1[:],
                                               op0=OP.mult, op1=OP.add)
                nc.vector.scalar_tensor_tensor(P10[:, o + 3 : o + 4], csig[:],
                                               -128.0, f2[:],
                                               op0=OP.mult, op1=OP.add)

            # ---- grouped coefficient tables [128, 32] ----
            grp = {}
            for name, t_idx in (("ax", 2), ("bx", 0), ("ex", 1), ("ey", 3),
                                ("bxn", 4)):
                g_ps = psum.tile([128, 32], f32, space="PSUM")
                for m in range(2):
                    rhs = persist.tile([128, 32], f32, tag="grp_rhs")
                    nc.vector.tensor_scalar(
                        rhs[:], selmask_t[:, m * 32 : m * 32 + 32],
                        P10[:, m * 5 + t_idx : m * 5 + t_idx + 1], None,
                        op0=OP.mult)
                    nc.tensor.matmul(g_ps[:],
                                     lhsT=sel_lhsT_t[:, m * 128 : m * 128 + 128],
                                     rhs=rhs[:], start=(m == 0), stop=(m == 1))
                g_sb = persist.tile([128, 32], f32, tag=f"grp_{name}")
                nc.vector.tensor_copy(g_sb[:], g_ps[:])
                grp[name] = g_sb

            # ---- per-pair anchor gather index (wrapped layout) for call j ----
            def compute_idx(j):
                kk = ds(j, 1)
                ax_c, bx_c = grp["ax"][:, kk], grp["bx"][:, kk]
                bxn_c, ex_c = grp["bxn"][:, kk], grp["ex"][:, kk]
                ey_c = grp["ey"][:, kk]

                def coord_floors(iw_t, ih_t, sa, sb, se, te_tag, to_tag):
                    te = widxp.tile([128, 512], f32, tag=te_tag, name="te")
                    to = widxp.tile([128, 512], f32, tag=to_tag, name="to")
                    fr = widxp.tile([128, 512], f32, tag="wsc", name="fr")
                    nc.vector.tensor_scalar(te[:], iw_t[:], sa, None,
                                            op0=OP.mult)
                    nc.vector.scalar_tensor_tensor(te[:], ih_t[:], sb, te[:],
                                                   op0=OP.mult, op1=OP.add)
                    nc.vector.tensor_scalar(te[:], te[:], se, None, op0=OP.add)
                    nc.vector.tensor_scalar(to[:], te[:], sb, None, op0=OP.add)
                    for q in (te, to):
                        nc.vector.tensor_scalar(q[:], q[:], 0.0, 127.0,
                                                op0=OP.max, op1=OP.min)
                        # floor via round-magic: fr=round(q); fr=q-fr (+1 if
                        # negative) = frac; q -= frac
                        nc.vector.tensor_scalar(fr[:], q[:], 12582912.0,
                                                -12582912.0,
                                                op0=OP.add, op1=OP.add)
                        nc.vector.tensor_tensor(fr[:], q[:], fr[:],
                                                op=OP.subtract)
                        nc.vector.scalar_tensor_tensor(fr[:], fr[:], 0.0,
                                                       fr[:], op0=OP.is_lt,
                                                       op1=OP.add)
                        nc.vector.tensor_tensor(q[:], q[:], fr[:],
                                                op=OP.subtract)
                    return te, to

                x0e, x0o = coord_floors(iww_t, ihw_t, ax_c, bx_c, ex_c,
                                        "wa", "wb")
                nc.vector.tensor_tensor(x0e[:], x0e[:], x0o[:], op=OP.min)
                # fl = floor(axm/2); block index jb = aym*64 + fl
                fl = widxp.tile([128, 512], f32, tag="wb", name="fl")
                fr2 = widxp.tile([128, 512], f32, tag="wsc", name="fr2")
                nc.vector.tensor_scalar(fl[:], x0e[:], 0.5, None, op0=OP.mult)
                nc.vector.tensor_scalar(fr2[:], fl[:], 12582912.0, -12582912.0,
                                        op0=OP.add, op1=OP.add)
                nc.vector.tensor_tensor(fr2[:], fl[:], fr2[:], op=OP.subtract)
                nc.vector.scalar_tensor_tensor(fr2[:], fr2[:], 0.0, fr2[:],
                                               op0=OP.is_lt, op1=OP.add)
                nc.vector.tensor_tensor(fl[:], fl[:], fr2[:], op=OP.subtract)
                y0e, y0o = coord_floors(iww_t, ihw_t, bxn_c, ax_c, ey_c,
                                        "wa", "wc")
                nc.vector.tensor_tensor(y0e[:], y0e[:], y0o[:], op=OP.min)
                nc.vector.scalar_tensor_tensor(fl[:], y0e[:], 64.0, fl[:],
                                               op0=OP.mult, op1=OP.add)
                b32 = widxp.tile([128, 512], i32, tag="wc", name="b32")
                nc.vector.tensor_copy(b32[:], fl[:])
                bt_ = bi16p.tile([128, 512], i16, tag="bi16")
                nc.vector.tensor_copy(bt_[:], b32[:])
                return bt_

            bi16_tiles = [None] * (CALLS + 1)
            bi16_tiles[0] = compute_idx(0)

            x_engs = [nc.scalar, nc.sync]

            # 16-lane bf16-singles source load for call j; the first 11 lanes
            # ride the SWDGE (gpsimd) queue -- dispatched one call AHEAD so
            # their Pool-queue slot lands between gather streams.
            def load_S(j):
                S = srcp.tile([128, NPIX // 2], f32, tag="S", name="S")
                Sb = S[:].bitcast(bf16)
                for li, (r, cx) in enumerate(LANES):
                    sig = 128 * r + cx
                    eng = (nc.gpsimd if li < 8
                           else (nc.sync if li % 2 == 0 else nc.scalar))
                    eng.dma_start(
                        Sb[li::16, :],
                        fmb_b[ds(8 * j, 8), ds(sig, NPIX)],
                    )
                return S

            S_tiles = [None] * CALLS
            S_tiles[0] = load_S(0)

            # ================= main loop =================
            for k in range(CALLS):
                kk = ds(k, 1)
                if k + 1 < CALLS:
                    S_tiles[k + 1] = load_S(k + 1)

                # ---- indices for the NEXT call (gather k+1 never waits) ----
                if k + 1 < CALLS:
                    bi16_tiles[k + 1] = compute_idx(k + 1)
                S = S_tiles[k]

                bi16 = bi16_tiles[k]
                ax_c, bx_c = grp["ax"][:, kk], grp["bx"][:, kk]
                bxn_c, ex_c = grp["bxn"][:, kk], grp["ex"][:, kk]
                ey_c = grp["ey"][:, kk]

                for sc in range(SC):
                    P = [prcp.tile([128, SCW], f32, tag=f"P{l}",
                                   name=f"P{l}")
                         for l in range(16)]
                    # ---- gather 2 chunks + lane extraction ----
                    for cc in range(2):
                        c = sc * 2 + cc
                        G = gathp.tile([128, CHUNK], f32, tag="G")
                        nc.gpsimd.ap_gather(
                            G[:].bitcast(bf16), S[:].bitcast(bf16),
                            bi16[:, ds(c * 128, 128)],
                            channels=128, num_elems=NPIX // 2, d=2,
                            num_idxs=CHUNK)
                        for li in range(16):
                            x_engs[li % 2].dma_start(
                                P[li][:, ds(cc * 128, 128)],
                                G[:][li::16, :]
                                .rearrange("g (r w) -> g r w", r=16),
                            )

                    # ---- per-pixel coords/fracs/offsets in P_rc layout ----
                    ss = ds(sc * SCW, SCW)
                    iwp_s, ihp_s = iwp_t[:, ss], ihp_t[:, ss]

                    def coord_full(sa, sb, se, tags):
                        te = wsp.tile([128, SCW], f32, tag=tags[0])
                        to = wsp.tile([128, SCW], f32, tag=tags[1])
                        fe = wsp.tile([128, SCW], f32, tag=tags[2])
                        fo = wsp.tile([128, SCW], f32, tag=tags[3])
                        nc.vector.tensor_scalar(te[:], iwp_s, sa, None,
                                                op0=OP.mult)
                        nc.vector.scalar_tensor_tensor(te[:], ihp_s, sb, te[:],
                                                       op0=OP.mult, op1=OP.add)
                        nc.vector.tensor_scalar(te[:], te[:], se, None,
                                                op0=OP.add)
                        nc.vector.tensor_scalar(to[:], te[:], sb, None,
                                                op0=OP.add)
                        for q, fq in ((te, fe), (to, fo)):
                            nc.vector.tensor_scalar(q[:], q[:], 0.0, 127.0,
                                                    op0=OP.max, op1=OP.min)
                            nc.vector.tensor_scalar(fq[:], q[:], 12582912.0,
                                                    -12582912.0,
                                                    op0=OP.add, op1=OP.add)
                            nc.vector.tensor_tensor(fq[:], q[:], fq[:],
                                                    op=OP.subtract)
                            nc.vector.scalar_tensor_tensor(fq[:], fq[:], 0.0,
                                                           fq[:],
                                                           op0=OP.is_lt,
                                                           op1=OP.add)
                            nc.vector.tensor_tensor(q[:], q[:], fq[:],
                                                    op=OP.subtract)
                        return te, to, fe, fo

                    x0e, x0o, fxe, fxo = coord_full(
                        ax_c, bx_c, ex_c, ("px0", "px1", "pfx0", "pfx1"))
                    y0e, y0o, fye, fyo = coord_full(
                        bxn_c, ax_c, ey_c, ("py0", "py1", "pfy0", "pfy1"))
                    axm = wsp.tile([128, SCW], f32, tag="paxm")
                    aym = wsp.tile([128, SCW], f32, tag="paym")
                    nc.vector.tensor_tensor(axm[:], x0e[:], x0o[:], op=OP.min)
                    nc.vector.tensor_tensor(aym[:], y0e[:], y0o[:], op=OP.min)
                    # par = axm mod 2 (block-misalignment of the d=2 gather)
                    par = wsp.tile([128, SCW], f32, tag="ppar")
                    pt = wsp.tile([128, SCW], f32, tag="ppt")
                    nc.vector.tensor_scalar(pt[:], axm[:], 0.5, None,
                                            op0=OP.mult)
                    nc.vector.tensor_scalar(par[:], pt[:], 12582912.0,
                                            -12582912.0, op0=OP.add,
                                            op1=OP.add)
                    nc.vector.tensor_tensor(par[:], pt[:], par[:],
                                            op=OP.subtract)
                    nc.vector.scalar_tensor_tensor(par[:], par[:], 0.0,
                                                   par[:], op0=OP.is_lt,
                                                   op1=OP.add)
                    nc.vector.tensor_scalar(par[:], par[:], 2.0, None,
                                            op0=OP.mult)
                    # deltas: dx in 0..2 (+par -> 0..3), dy in 0..2
                    dups = []
                    for dn, (pos, anc, addpar) in enumerate(
                            ((x0e, axm, True), (x0o, axm, True),
                             (y0e, aym, False), (y0o, aym, False))):
                        nc.vector.tensor_tensor(pos[:], pos[:], anc[:],
                                                op=OP.subtract)
                        if addpar:
                            nc.vector.tensor_tensor(pos[:], pos[:], par[:],
                                                    op=OP.add)
                        dd = wsp.tile([128, 2 * SCW], bf16, tag=f"dd{dn}")
                        nc.vector.tensor_copy(dd[:, 0::2], pos[:])
                        nc.vector.tensor_copy(dd[:, 1::2], pos[:])
                        dups.append(dd)
                    dxe, dxo, dye, dyo = dups

                    Lb = [P[l][:].bitcast(bf16) for l in range(16)]

                    # ---- masked patch selection + bilinear, per class ----
                    for cls, (dxd, dyd, fx, fy, par) in enumerate(
                            ((dxe, dye, fxe, fye, 0),
                             (dxo, dyo, fxo, fyo, 1))):
                        Cr = []
                        Ct = wsp.tile([128, 2 * SCW], bf16, tag="paxm",
                                      name="Ct")
                        ctags = ("px0", "px1", "py0", "py1")
                        for r in range(4):
                            C = wsp.tile([128, 2 * SCW], bf16, tag=ctags[r],
                                         name=f"C{r}")
                            nc.vector.scalar_tensor_tensor(
                                C[:], dxd[:], 0.0, Lb[r * 4 + 0],
                                op0=OP.is_equal, op1=OP.mult)
                            for cx in (1, 2, 3):
                                nc.vector.scalar_tensor_tensor(
                                    Ct[:], dxd[:], float(cx), Lb[r * 4 + cx],
                                    op0=OP.is_equal, op1=OP.mult)
                                nc.vector.tensor_tensor(C[:], C[:], Ct[:],
                                                        op=OP.add)
                            Cr.append(C)
                        PK = []
                        ktags = ("paym", "K1")
                        for tap in range(2):
                            Kt = wsp.tile([128, 2 * SCW], bf16,
                                          tag=ktags[tap], name=f"K{tap}")
                            nc.vector.scalar_tensor_tensor(
                                Kt[:], dyd[:], 0.0, Cr[tap][:],
                                op0=OP.is_equal, op1=OP.mult)
                            for rr in (1, 2):
                                nc.vector.scalar_tensor_tensor(
                                    Ct[:], dyd[:], float(rr), Cr[tap + rr][:],
                                    op0=OP.is_equal, op1=OP.mult)
                                nc.vector.tensor_tensor(Kt[:], Kt[:], Ct[:],
                                                        op=OP.add)
                            PK.append(Kt)
                        # x-lerp from packed bf16 pairs, then y-lerp
                        Rs = []
                        for tap in range(2):
                            lo = PK[tap][:, 0::2]
                            hi = PK[tap][:, 1::2]
                            Rt = wsp.tile([128, SCW], f32, tag=f"R{tap}")
                            nc.vector.tensor_tensor(Rt[:], hi, lo,
                                                    op=OP.subtract)
                            nc.vector.tensor_tensor(Rt[:], Rt[:], fx[:],
                                                    op=OP.mult)
                            nc.vector.tensor_tensor(Rt[:], Rt[:], lo,
                                                    op=OP.add)
                            Rs.append(Rt)
                        O = outp.tile([128, SCW], bf16, tag=f"O{cls}")
                        nc.vector.tensor_tensor(Rs[1][:], Rs[1][:], Rs[0][:],
                                                op=OP.subtract)
                        nc.vector.tensor_tensor(Rs[1][:], Rs[1][:], fy[:],
                                                op=OP.mult)
                        nc.vector.tensor_tensor(O[:], Rs[1][:], Rs[0][:],
                                                op=OP.add)
                        # ---- output DMA: rows h = 2v+par, contiguous w ----
                        for cc in range(2):
                            x_engs[(cls + cc) % 2].dma_start(
                                out5[ds(8 * k, 8), sc * 2 + cc,
                                     slice(None), par, slice(None)],
                                O[:, ds(cc * 128, 128)],
                            )

    nc.compile()
    return nc


def _prepare_in_maps(feature_map, para_code, W1, b1, Ws, bs, Wr, br, Wt, bt):
    import ml_dtypes

    consts = _host_constants()
    Wt_re = np.concatenate([Wt[:, 0::2], Wt[:, 1::2]], axis=1)
    bt_re = np.concatenate([bt[0::2], bt[1::2]])
    common = dict(
        w1=_mm_layout(W1, 256), ws=_mm_layout(Ws, 256), wr=_mm_layout(Wr, 256),
        wt=_mm_layout(Wt_re, 512),
        b1=_col2(b1), bs=_col2(bs), br=_col2(br),
        bt=np.ascontiguousarray(bt_re.reshape(4, 128).T),
        **consts,
    )
    common = {k: np.ascontiguousarray(v, dtype=np.float32)
              for k, v in common.items()}
    in_maps = []
    for i in range(NCORES):
        flat = np.ascontiguousarray(feature_map[i].reshape(D, NPIX),
                                    dtype=np.float32)
        fmb = np.zeros((D, CH_PITCHB), dtype=ml_dtypes.bfloat16)
        fmb[:, :NPIX] = flat.astype(ml_dtypes.bfloat16)
        m = dict(common)
        m["fmb"] = np.ascontiguousarray(fmb).view(np.float32).reshape(-1)
        m["pc"] = _col2(para_code[i])
        in_maps.append(m)
    return in_maps


def _run(inputs, trace=False):
    from concourse.bass_utils import run_bass_kernel_spmd

    if "nc" not in _GRAPH_CACHE:
        _GRAPH_CACHE["nc"] = _build()
    nc = _GRAPH_CACHE["nc"]
    in_maps = _prepare_in_maps(**inputs)
    res = run_bass_kernel_spmd(nc, in_maps, core_ids=list(range(NCORES)),
                               trace=trace)
    out = np.stack([
        np.asarray(res.results[i]["out"]).astype(np.float32).reshape(D, H, W)
        for i in range(NCORES)
    ])
    return out, res


def kernel(**inputs) -> np.ndarray:
    out, _ = _run(inputs, trace=False)
    return out
